# revision 23
# baseline (speedup 1.0000x reference)
"""MANN LSTM cell (memory-augmented NN) Trainium2 kernel.

Strategy: shard the memory bank [M, U] and all [B, M] addressing tensors
along the M (slot) axis across 8 NeuronCores. The tiny LSTM gate GEMMs are
column-sharded (each core computes a 128-wide slice of h/c, AllGather).
Similarity/softmax/usage/write are slot-parallel with three tiny
collectives (softmax denominator AllReduce, usage-stats AllGather, final
r AllReduce).

Self-contained: hardcodes B=32, U=1024, M=65536, 8 cores.
"""

import numpy as np

B = 32
U = 1024
M_FULL = 65536
NCORES = 8
KT = U // 128           # 8 k-tiles over the U (contraction) axis
BIG = 3.0e38


def build_nc(M=M_FULL, ncores=NCORES):
    import concourse.bass as bass
    import concourse.bacc as bacc
    import concourse.mybir as mybir
    import concourse.tile as tile

    f32 = mybir.dt.float32
    f32r = mybir.dt.float32r
    bf16 = mybir.dt.bfloat16
    i32 = mybir.dt.int32
    u32 = mybir.dt.uint32
    AF = mybir.ActivationFunctionType
    ALU = mybir.AluOpType
    AX = mybir.AxisListType

    Ms = M // ncores          # slots per core (8192)
    Us = U // ncores          # gate-slice width per core (128)
    NT = Ms // 128            # m-tiles per core (64)
    SUP = 4                   # m-tiles per supertile
    NST = (NT + SUP - 1) // SUP

    rg = [list(range(ncores))]

    nc = bacc.Bacc(None, num_devices=ncores)

    # ---------------- I/O ----------------
    x_in = nc.dram_tensor("x", [B, U], f32, kind="ExternalInput")
    h_tm1_in = nc.dram_tensor("h_tm1", [B, U], f32, kind="ExternalInput")
    r_tm1_in = nc.dram_tensor("r_tm1", [B, U], f32, kind="ExternalInput")
    c_tm1_in = nc.dram_tensor("c_tm1_s", [B, Us], f32, kind="ExternalInput")
    bank_in = nc.dram_tensor("bank_s", [Ms, U], f32, kind="ExternalInput")
    wu_in = nc.dram_tensor("wu_s", [B, Ms], f32, kind="ExternalInput")
    wlu_in = nc.dram_tensor("wlu_s", [B, Ms], f32, kind="ExternalInput")
    wrp_in = nc.dram_tensor("wrp_s", [B, Ms], f32, kind="ExternalInput")
    k_in = nc.dram_tensor("k_s", [U, 4 * Us], f32, kind="ExternalInput")
    rk_in = nc.dram_tensor("rk_s", [U, 5 * Us], f32, kind="ExternalInput")
    bias_in = nc.dram_tensor("bias_s", [1, 4 * Us], f32, kind="ExternalInput")
    wg_in = nc.dram_tensor("wg", [1, 1], f32, kind="ExternalInput")
    m0f_in = nc.dram_tensor("m0f", [128, 1], f32, kind="ExternalInput")
    iota_in = nc.dram_tensor("iota_g", [128, Ms // 128], f32,
                             kind="ExternalInput")

    r_out = nc.dram_tensor("r_out", [B, U], f32, kind="ExternalOutput")
    h_out = nc.dram_tensor("h_out", [B, U], f32, kind="ExternalOutput")
    c_out = nc.dram_tensor("c_out", [B, U], f32, kind="ExternalOutput")
    mem_out = nc.dram_tensor("mem_out", [Ms, U], f32, kind="ExternalOutput")
    wu_out = nc.dram_tensor("wu_out", [B, Ms], f32, kind="ExternalOutput")
    wlu_out = nc.dram_tensor("wlu_out", [B, Ms], f32, kind="ExternalOutput")
    wr_out = nc.dram_tensor("wr_out", [B, Ms], f32, kind="ExternalOutput")

    id32_t = nc.inline_tensor(np.eye(32, dtype=np.float32), name="id32c")
    id128_t = nc.inline_tensor(np.eye(128, dtype=np.float32), name="id128c")
    one11_t = nc.inline_tensor(np.ones((1, 1), dtype=np.float32), name="one11c")

    with tile.TileContext(nc) as tc, \
            tc.tile_pool(name="const", bufs=1) as cst, \
            tc.tile_pool(name="persist", bufs=1) as per, \
            tc.tile_pool(name="dram", bufs=1, space="DRAM") as dram:

        # ---------------- constants ----------------
        id32 = cst.tile([32, 32], f32)
        nc.sync.dma_start(out=id32, in_=id32_t[:, :])
        id32b = cst.tile([32, 32], bf16)
        nc.vector.tensor_copy(out=id32b, in_=id32)
        id128 = cst.tile([128, 128], f32)
        nc.sync.dma_start(out=id128, in_=id128_t[:, :])
        id128b = cst.tile([128, 128], bf16)
        nc.vector.tensor_copy(out=id128b, in_=id128)
        one11 = cst.tile([1, 1], f32)
        nc.sync.dma_start(out=one11, in_=one11_t[:, :])
        m0f = cst.tile([128, 1], f32)
        nc.sync.dma_start(out=m0f, in_=m0f_in[:, :])

        # global slot index of (partition p, tile t): m0 + 128*t + p
        iota_g = cst.tile([128, NT], f32)
        nc.sync.dma_start(out=iota_g, in_=iota_in[:, :])

        # ---------------- DRAM bounce buffers ----------------
        hc_in = dram.tile([B, 2 * Us], f32)
        hc_ag = dram.tile([ncores * B, 2 * Us], f32)
        se_in = dram.tile([B, 1], f32)
        se_ar = dram.tile([B, 1], f32)
        gidx_d = dram.tile([1, 1], f32)
        st2_in = dram.tile([4, 32], f32)
        st2_ag = dram.tile([4 * ncores, 32], f32)
        r_in = dram.tile([B, U], f32)
        r_ar = dram.tile([B, U], f32)

        # ---------------- persistent SBUF ----------------
        h_sb = per.tile([B, U], f32)
        c_sb = per.tile([B, U], f32)
        hTn = per.tile([128, KT * 32], bf16)       # normalized h, transposed
        wr_sb = per.tile([B, Ms], f32)             # exp(sim) -> wr
        wu_new = per.tile([B, Ms], f32)
        ww_sb = per.tile([B, Ms], f32)
        wrT = per.tile([128, NT * 32], f32)        # wr transposed k-tiles
        sump = per.tile([B, NST], f32)             # per-supertile exp sums
        keep_sb = per.tile([128, NT], f32)
        nth_col = per.tile([B, 1], f32)
        r_sb = per.tile([B, U], f32)
        ww_bf = per.tile([B, Ms], bf16)
        h_bf = per.tile([B, U], bf16)

        # ================= Phase A: LSTM gates (column-sharded) =========
        with tc.tile_pool(name="gates", bufs=1) as gp, \
                tc.tile_pool(name="gtmp", bufs=1) as gt, \
                tc.tile_pool(name="gpsum", bufs=2, space="PSUM") as gps:
            ksb = gp.tile([128, KT, 4 * Us], f32)
            nc.sync.dma_start(out=ksb, in_=k_in[:, :].rearrange(
                "(kt p) n -> p kt n", p=128))
            rksb = gp.tile([128, KT, 5 * Us], f32)
            nc.sync.dma_start(out=rksb, in_=rk_in[:, :].rearrange(
                "(kt p) n -> p kt n", p=128))
            bias_b = gp.tile([B, 4 * Us], f32)
            nc.sync.dma_start(out=bias_b, in_=bass.AP(
                tensor=bias_in, offset=0, ap=[[0, B], [1, 4 * Us]]))
            x_sb = gp.tile([B, U], f32)
            nc.sync.dma_start(out=x_sb, in_=x_in[:, :])
            ho_sb = gp.tile([B, U], f32)
            nc.sync.dma_start(out=ho_sb, in_=h_tm1_in[:, :])
            ro_sb = gp.tile([B, U], f32)
            nc.sync.dma_start(out=ro_sb, in_=r_tm1_in[:, :])
            co_sb = gp.tile([B, Us], f32)
            nc.sync.dma_start(out=co_sb, in_=c_tm1_in[:, :])

            # transposes of x, h_tm1, r_tm1 -> [128, KT*32]
            xT = gp.tile([128, KT * 32], f32)
            hT = gp.tile([128, KT * 32], f32)
            rT = gp.tile([128, KT * 32], f32)
            for src, dstT in ((x_sb, xT), (ho_sb, hT), (ro_sb, rT)):
                for g in range(KT // 4):
                    pT = gps.tile([128, 128], f32, tag="pT")
                    for q in range(4):
                        k = 4 * g + q
                        nc.tensor.transpose(
                            pT[:, 32 * q:32 * q + 32],
                            src[:, 128 * k:128 * k + 128], id32)
                    nc.vector.tensor_copy(
                        out=dstT[:, 128 * g:128 * g + 128], in_=pT)

            # z + hz  [B, 4*Us]
            zps = gps.tile([B, 4 * Us], f32, tag="zps")
            for k in range(KT):
                nc.tensor.matmul(
                    zps, xT[:, 32 * k:32 * k + 32], ksb[:, k, :],
                    start=(k == 0), stop=False)
            for k in range(KT):
                nc.tensor.matmul(
                    zps, hT[:, 32 * k:32 * k + 32], rksb[:, k, 0:4 * Us],
                    start=False, stop=(k == KT - 1))
            rips = gps.tile([B, Us], f32, tag="rips")
            for k in range(KT):
                nc.tensor.matmul(
                    rips, rT[:, 32 * k:32 * k + 32],
                    rksb[:, k, 4 * Us:5 * Us],
                    start=(k == 0), stop=(k == KT - 1))

            pre = gt.tile([B, 4 * Us], f32)
            nc.vector.tensor_add(out=pre, in0=zps, in1=bias_b)

            zi = pre[:, 0 * Us:1 * Us]
            zf = pre[:, 1 * Us:2 * Us]
            zc = pre[:, 2 * Us:3 * Us]
            zo = pre[:, 3 * Us:4 * Us]

            half_b = gt.tile([B, 1], f32)
            nc.vector.memset(half_b, 0.5)
            # i = hard_sigmoid(zi + r_i)
            i_g = gt.tile([B, Us], f32)
            nc.vector.tensor_add(out=i_g, in0=zi, in1=rips)
            nc.scalar.activation(out=i_g, in_=i_g, func=AF.Relu,
                                 bias=half_b, scale=0.2)
            nc.vector.tensor_scalar_min(i_g, i_g, 1.0)
            # f, o = hard_sigmoid(zf), hard_sigmoid(zo)
            f_g = gt.tile([B, Us], f32)
            nc.scalar.activation(out=f_g, in_=zf, func=AF.Relu,
                                 bias=half_b, scale=0.2)
            nc.vector.tensor_scalar_min(f_g, f_g, 1.0)
            o_g = gt.tile([B, Us], f32)
            nc.scalar.activation(out=o_g, in_=zo, func=AF.Relu,
                                 bias=half_b, scale=0.2)
            nc.vector.tensor_scalar_min(o_g, o_g, 1.0)
            # c = f*c_tm1 + i*tanh(zc)
            tzc = gt.tile([B, Us], f32)
            nc.scalar.activation(out=tzc, in_=zc, func=AF.Tanh)
            hc_sb = gt.tile([B, 2 * Us], f32)
            c_slice = hc_sb[:, Us:2 * Us]
            nc.vector.tensor_mul(out=c_slice, in0=i_g, in1=tzc)
            fc = gt.tile([B, Us], f32)
            nc.vector.tensor_mul(out=fc, in0=f_g, in1=co_sb)
            nc.vector.tensor_add(out=c_slice, in0=c_slice, in1=fc)
            # h = o * tanh(c)
            tc_ = gt.tile([B, Us], f32)
            nc.scalar.activation(out=tc_, in_=c_slice, func=AF.Tanh)
            nc.vector.tensor_mul(out=hc_sb[:, 0:Us], in0=o_g, in1=tc_)

            # AllGather h||c
            nc.sync.dma_start(out=hc_in[:, :], in_=hc_sb)
            nc.gpsimd.collective_compute(
                "AllGather", ALU.bypass, replica_groups=rg,
                ins=[hc_in[:, :].opt()], outs=[hc_ag[:, :].opt()])
            hc_view = hc_ag[:, :].rearrange("(s b) n -> b s n", b=B)
            nc.sync.dma_start(
                out=h_sb.rearrange("b (s n) -> b s n", s=ncores),
                in_=hc_view[:, :, 0:Us])
            nc.sync.dma_start(
                out=c_sb.rearrange("b (s n) -> b s n", s=ncores),
                in_=hc_view[:, :, Us:2 * Us])
            nc.sync.dma_start(out=h_out[:, :], in_=h_sb)
            nc.sync.dma_start(out=c_out[:, :], in_=c_sb)
            nc.vector.tensor_copy(out=h_bf, in_=h_sb)

            # normalized h, transposed, bf16 (x_sb is dead: reuse as scratch)
            ssh = gt.tile([B, 1], f32)
            nc.scalar.activation(out=x_sb, in_=h_sb, func=AF.Square,
                                 accum_out=ssh)
            nc.vector.tensor_scalar_max(ssh, ssh, 1e-12)
            nc.scalar.activation(out=ssh, in_=ssh, func=AF.Sqrt)
            nc.vector.reciprocal(out=ssh, in_=ssh)
            hn = gt.tile([B, U], bf16)
            nc.vector.tensor_scalar_mul(hn, h_sb, ssh)
            for g in range(KT // 4):
                pT2 = gps.tile([128, 128], bf16, tag="pTb")
                for q in range(4):
                    k = 4 * g + q
                    nc.tensor.transpose(
                        pT2[:, 32 * q:32 * q + 32],
                        hn[:, 128 * k:128 * k + 128], id32b)
                nc.vector.tensor_copy(
                    out=hTn[:, 128 * g:128 * g + 128], in_=pT2)

            # sg = sigmoid(write_gate) on a DMA-partition-broadcast copy
            wg_b = gt.tile([B, 1], f32)
            nc.sync.dma_start(out=wg_b, in_=bass.AP(
                tensor=wg_in, offset=0, ap=[[0, B], [1, 1]]))
            sg_b = per.tile([B, 1], f32)
            nc.scalar.activation(out=sg_b, in_=wg_b, func=AF.Sigmoid)
            omsg_b = per.tile([B, 1], f32)
            nc.vector.tensor_scalar(
                out=omsg_b, in0=sg_b, scalar1=-1.0, scalar2=1.0,
                op0=ALU.mult, op1=ALU.add)

        # ww = sg*wr_prev + (1-sg)*wlu  (chunked loads)
        with tc.tile_pool(name="wwp", bufs=2) as wp:
            CH = min(2048, Ms)
            for ci in range(Ms // CH):
                sl = slice(ci * CH, (ci + 1) * CH)
                wlu_c = wp.tile([B, CH], f32, tag="wluc")
                nc.sync.dma_start(out=wlu_c, in_=wlu_in[:, sl])
                wrp_c = wp.tile([B, CH], f32, tag="wrpc")
                nc.sync.dma_start(out=wrp_c, in_=wrp_in[:, sl])
                nc.vector.tensor_scalar_mul(ww_sb[:, sl], wlu_c, omsg_b)
                nc.vector.scalar_tensor_tensor(
                    out=ww_sb[:, sl], in0=wrp_c, scalar=sg_b,
                    in1=ww_sb[:, sl], op0=ALU.mult, op1=ALU.add)
                nc.vector.tensor_copy(out=ww_bf[:, sl], in_=ww_sb[:, sl])

        # ================= Phase B: pass 1 (sim + exp) ==================
        with tc.tile_pool(name="p1", bufs=3) as p1, \
                tc.tile_pool(name="p1n", bufs=SUP + 1) as p1n, \
                tc.tile_pool(name="p1s", bufs=2) as p1s, \
                tc.tile_pool(name="p1ps", bufs=2, space="PSUM") as p1ps, \
                tc.tile_pool(name="p1sim", bufs=2, space="PSUM") as p1sim:
            for st in range(NST):
                q_n = min(SUP, NT - st * SUP)
                norm_tiles = []
                for q in range(q_n):
                    t = st * SUP + q
                    bk = p1.tile([128, U], f32, tag="bk")
                    nc.sync.dma_start(
                        out=bk, in_=bank_in[128 * t:128 * t + 128, :])
                    sq = p1.tile([128, U], f32, tag="sq")
                    ss = p1.tile([128, 1], f32, tag="ss", bufs=SUP + 2)
                    nc.scalar.activation(out=sq, in_=bk, func=AF.Square,
                                         accum_out=ss)
                    nc.vector.tensor_scalar_max(ss, ss, 1e-12)
                    nc.scalar.activation(out=ss, in_=ss, func=AF.Sqrt)
                    nc.vector.reciprocal(out=ss, in_=ss)
                    bn = p1n.tile([128, U], bf16, tag="bn")
                    nc.vector.tensor_scalar_mul(bn, bk, ss)
                    norm_tiles.append(bn)
                bT = p1s.tile([128, KT, 128 * SUP], bf16, tag="bT")
                for j in range(KT):
                    pj = p1ps.tile([128, 128 * SUP], bf16, tag="pj")
                    for q in range(q_n):
                        nc.tensor.transpose(
                            pj[:, 128 * q:128 * q + 128],
                            norm_tiles[q][:, 128 * j:128 * j + 128],
                            id128b)
                    nc.vector.tensor_copy(out=bT[:, j, 0:128 * q_n],
                                          in_=pj[:, 0:128 * q_n])
                simp = p1sim.tile([B, 128 * SUP], f32, tag="simp")
                for j in range(KT):
                    nc.tensor.matmul(
                        simp[:, 0:128 * q_n],
                        hTn[:, 32 * j:32 * j + 32],
                        bT[:, j, 0:128 * q_n],
                        start=(j == 0), stop=(j == KT - 1))
                nc.scalar.activation(
                    out=wr_sb[:, st * SUP * 128:st * SUP * 128 + 128 * q_n],
                    in_=simp[:, 0:128 * q_n], func=AF.Exp,
                    accum_out=sump[:, st:st + 1])

        # ================= Phase C: softmax denominator =================
        sume = per.tile([B, 1], f32)
        nc.vector.tensor_reduce(out=sume, in_=sump, axis=AX.X, op=ALU.add)
        nc.sync.dma_start(out=se_in[:, :], in_=sume)
        nc.gpsimd.collective_compute(
            "AllReduce", ALU.add, replica_groups=rg,
            ins=[se_in[:, :].opt()], outs=[se_ar[:, :].opt()])
        gsum = per.tile([B, 1], f32)
        nc.sync.dma_start(out=gsum, in_=se_ar[:, :])
        nc.vector.reciprocal(out=gsum, in_=gsum)
        nc.vector.tensor_scalar_mul(wr_sb, wr_sb, gsum)
        nc.sync.dma_start(out=wr_out[:, :], in_=wr_sb)

        # wr transposed (k-tiles for pass-2 r matmul)
        with tc.tile_pool(name="wtps", bufs=2, space="PSUM") as wtps:
            for g in range(NT // 4):
                pw = wtps.tile([128, 128], f32, tag="pw")
                for q in range(4):
                    t = 4 * g + q
                    nc.tensor.transpose(
                        pw[:, 32 * q:32 * q + 32],
                        wr_sb[:, 128 * t:128 * t + 128], id32)
                nc.vector.tensor_copy(out=wrT[:, 128 * g:128 * g + 128],
                                      in_=pw)

        # ================= Phase D: usage stats =========================
        with tc.tile_pool(name="pd", bufs=1) as pd, \
                tc.tile_pool(name="pdp", bufs=1, space="PSUM") as pdp:
            CH = min(2048, Ms)
            nwu = pd.tile([B, Ms], f32)
            for ci in range(Ms // CH):
                sl = slice(ci * CH, (ci + 1) * CH)
                wu_c = pd.tile([B, CH], f32, tag="wuc", bufs=3)
                nc.sync.dma_start(out=wu_c, in_=wu_in[:, sl])
                nc.vector.scalar_tensor_tensor(
                    out=wu_new[:, sl], in0=wu_c, scalar=0.5,
                    in1=wr_sb[:, sl], op0=ALU.mult, op1=ALU.add)
                nc.vector.tensor_add(
                    out=wu_new[:, sl], in0=wu_new[:, sl], in1=ww_sb[:, sl])
                nc.vector.tensor_scalar_mul(nwu[:, sl], wu_new[:, sl], -1.0)
            nc.sync.dma_start(out=wu_out[:, :], in_=wu_new)

            mx8 = pd.tile([B, 8], f32)
            nc.vector.max(out=mx8, in_=nwu)
            ix8 = pd.tile([B, 8], u32)
            nc.vector.max_index(out=ix8, in_max=mx8, in_values=nwu)

            stats = pd.tile([B, 4], f32)
            nc.vector.memset(stats, 0.0)
            nc.vector.tensor_scalar_mul(stats[:, 0:1], mx8[:, 0:1], -1.0)
            nc.vector.tensor_scalar_mul(stats[:, 1:2], mx8[:, 1:2], -1.0)
            ixf = pd.tile([B, 1], f32)
            nc.vector.tensor_copy(out=ixf, in_=ix8[:, 0:1])
            nc.vector.tensor_scalar_add(stats[:, 2:3], ixf, m0f[0:B])

            pst = pdp.tile([4, 32], f32)
            nc.tensor.transpose(pst, stats, id32)
            st_sb = pd.tile([4, 32], f32)
            nc.vector.tensor_copy(out=st_sb, in_=pst)
            nc.sync.dma_start(out=st2_in[:, :], in_=st_sb)
            nc.gpsimd.collective_compute(
                "AllGather", ALU.bypass, replica_groups=rg,
                ins=[st2_in[:, :].opt()], outs=[st2_ag[:, :].opt()])
            stg = pd.tile([1, 4 * ncores * 32], f32)
            nc.sync.dma_start(out=stg, in_=bass.AP(
                tensor=st2_ag.tensor, offset=st2_ag.offset,
                ap=[[0, 1], [1, 4 * ncores * 32]]))
            # layout: stg[0, s*128 + j*32 + b], j in {0:min1, 1:min2, 2:gidx}
            stv = stg.rearrange("p (s j b) -> p s j b", s=ncores, j=4)
            gmin1 = pd.tile([1, 32], f32)
            nc.vector.tensor_reduce(
                out=gmin1, in_=stv[:, :, 0:2, :].rearrange(
                    "p s j b -> p b s j"),
                axis=AX.XY, op=ALU.min)
            # second smallest: penalize all copies of the min, re-reduce
            pen = pd.tile([1, 2 * ncores, 32], f32)
            for s in range(ncores):
                for j in range(2):
                    eq = pd.tile([1, 32], f32, tag="eqt", bufs=4)
                    nc.vector.tensor_tensor(
                        out=eq, in0=stv[:, s, j, :], in1=gmin1,
                        op=ALU.is_equal)
                    nc.vector.scalar_tensor_tensor(
                        out=pen[:, 2 * s + j, :], in0=eq, scalar=BIG,
                        in1=stv[:, s, j, :], op0=ALU.mult, op1=ALU.add)
            gmin2 = pd.tile([1, 32], f32)
            nc.vector.tensor_reduce(
                out=gmin2, in_=pen.rearrange("p c b -> p b c"),
                axis=AX.X, op=ALU.min)
            # per-row global argmin (first occurrence), then min over rows
            pidx = pd.tile([1, ncores, 32], f32)
            nc.vector.memset(pidx, BIG)
            for s in range(ncores):
                eq2 = pd.tile([1, 32], u32, tag="eqti", bufs=4)
                nc.vector.tensor_tensor(
                    out=eq2, in0=stv[:, s, 0, :], in1=gmin1,
                    op=ALU.is_equal)
                nc.vector.copy_predicated(
                    out=pidx[:, s, :], mask=eq2, data=stv[:, s, 2, :])
            rowmin = pd.tile([1, 32], f32)
            nc.vector.tensor_reduce(
                out=rowmin, in_=pidx.rearrange("p s b -> p b s"),
                axis=AX.X, op=ALU.min)
            gidx = pd.tile([1, 1], f32)
            nc.vector.tensor_reduce(out=gidx, in_=rowmin, axis=AX.X,
                                    op=ALU.min)
            # nth_smallest -> per-partition column [B, 1]
            pnth = pdp.tile([32, 1], f32)
            nc.tensor.matmul(pnth, gmin2, one11, is_transpose=True)
            nc.vector.tensor_copy(out=nth_col, in_=pnth)
            # keep mask [128, NT]  (broadcast gidx via DRAM round-trip)
            nc.sync.dma_start(out=gidx_d[:, :], in_=gidx)
            idx_b = pd.tile([128, 1], f32)
            nc.sync.dma_start(out=idx_b, in_=bass.AP(
                tensor=gidx_d.tensor, offset=gidx_d.offset,
                ap=[[0, 128], [1, 1]]))
            nc.vector.tensor_scalar(
                out=keep_sb, in0=iota_g, scalar1=idx_b, scalar2=None,
                op0=ALU.not_equal)
            # wlu_new = (wu_new <= nth)
            nc.vector.tensor_scalar(
                out=nwu, in0=wu_new, scalar1=nth_col, scalar2=None,
                op0=ALU.is_le)
            nc.sync.dma_start(out=wlu_out[:, :], in_=nwu)

        # ================= Phase E: pass 2 (r + memory write) ===========
        with tc.tile_pool(name="p2", bufs=3) as p2, \
                tc.tile_pool(name="p2r", bufs=1, space="PSUM") as p2r, \
                tc.tile_pool(name="p2m", bufs=2, space="PSUM") as p2m:
            rps0 = p2r.tile([B, 512], f32, tag="rps0")
            rps1 = p2r.tile([B, 512], f32, tag="rps1")
            for t in range(NT):
                bk2 = p2.tile([128, U], f32, tag="bk2")
                nc.sync.dma_start(
                    out=bk2, in_=bank_in[128 * t:128 * t + 128, :])
                nc.tensor.matmul(
                    rps0, wrT[:, 32 * t:32 * t + 32], bk2[:, 0:512],
                    start=(t == 0), stop=(t == NT - 1))
                nc.tensor.matmul(
                    rps1, wrT[:, 32 * t:32 * t + 32], bk2[:, 512:1024],
                    start=(t == 0), stop=(t == NT - 1))
                mps = p2m.tile([128, U], f32, tag="mps")
                nc.tensor.matmul(
                    mps[:, 0:512],
                    ww_bf[:, 128 * t:128 * t + 128],
                    h_bf[:, 0:512], start=True, stop=True)
                nc.tensor.matmul(
                    mps[:, 512:1024],
                    ww_bf[:, 128 * t:128 * t + 128],
                    h_bf[:, 512:1024], start=True, stop=True)
                ot = p2.tile([128, U], f32, tag="ot")
                nc.vector.scalar_tensor_tensor(
                    out=ot, in0=bk2, scalar=keep_sb[:, t:t + 1], in1=mps,
                    op0=ALU.mult, op1=ALU.add)
                nc.sync.dma_start(
                    out=mem_out[128 * t:128 * t + 128, :], in_=ot)
            nc.vector.tensor_copy(out=r_sb[:, 0:512], in_=rps0)
            nc.vector.tensor_copy(out=r_sb[:, 512:1024], in_=rps1)
            nc.sync.dma_start(out=r_in[:, :], in_=r_sb)
            nc.gpsimd.collective_compute(
                "AllReduce", ALU.add, replica_groups=rg,
                ins=[r_in[:, :].opt()], outs=[r_ar[:, :].opt()])
            nc.sync.dma_start(out=r_out[:, :], in_=r_ar[:, :])

    nc.finalize()
    return nc


_NC_CACHE = {}


def _get_nc(M=M_FULL):
    if M not in _NC_CACHE:
        _NC_CACHE[M] = build_nc(M)
    return _NC_CACHE[M]


def make_in_maps(inputs, M=M_FULL, ncores=NCORES):
    """Shard full inputs into per-core input maps."""
    Ms = M // ncores
    Us = U // ncores
    x = np.ascontiguousarray(np.asarray(inputs["x"], dtype=np.float32))
    h_tm1 = np.ascontiguousarray(np.asarray(inputs["h_tm1"], np.float32))
    c_tm1 = np.asarray(inputs["c_tm1"], np.float32)
    r_tm1 = np.ascontiguousarray(np.asarray(inputs["r_tm1"], np.float32))
    bank = np.asarray(inputs["memory_bank"], np.float32)
    wu = np.asarray(inputs["wu"], np.float32)
    wlu = np.asarray(inputs["wlu"], np.float32)
    wrp = np.asarray(inputs["wr_prev"], np.float32)
    kern = np.asarray(inputs["kernel"], np.float32)
    rk = np.asarray(inputs["recurrent_kernel"], np.float32)
    bias = np.asarray(inputs["bias"], np.float32)
    wg = np.asarray(inputs["write_gate"], np.float32).reshape(1, 1)

    in_maps = []
    for s in range(ncores):
        sl = slice(s * Ms, (s + 1) * Ms)
        gsl = slice(s * Us, (s + 1) * Us)
        k_s = np.concatenate(
            [kern[:, g * U + s * Us:g * U + (s + 1) * Us] for g in range(4)],
            axis=1)
        rk_s = np.concatenate(
            [rk[:, g * U + s * Us:g * U + (s + 1) * Us] for g in range(5)],
            axis=1)
        bias_s = np.concatenate(
            [bias[g * U + s * Us:g * U + (s + 1) * Us] for g in range(4)]
        )[None, :]
        in_maps.append({
            "x": x,
            "h_tm1": h_tm1,
            "r_tm1": r_tm1,
            "c_tm1_s": np.ascontiguousarray(c_tm1[:, gsl]),
            "bank_s": np.ascontiguousarray(bank[sl, :]),
            "wu_s": np.ascontiguousarray(wu[:, sl]),
            "wlu_s": np.ascontiguousarray(wlu[:, sl]),
            "wrp_s": np.ascontiguousarray(wrp[:, sl]),
            "k_s": np.ascontiguousarray(k_s),
            "rk_s": np.ascontiguousarray(rk_s),
            "bias_s": np.ascontiguousarray(bias_s),
            "wg": wg,
            "m0f": np.full((128, 1), float(s * Ms), np.float32),
            "iota_g": (float(s * Ms)
                       + np.arange(Ms // 128, dtype=np.float32)[None, :] * 128
                       + np.arange(128, dtype=np.float32)[:, None]),
        })
    return in_maps


def assemble(results):
    r = results[0]["r_out"]
    h = results[0]["h_out"]
    c = results[0]["c_out"]
    mem = np.concatenate([res["mem_out"] for res in results], axis=0)
    wu_new = np.concatenate([res["wu_out"] for res in results], axis=1)
    wlu_new = np.concatenate([res["wlu_out"] for res in results], axis=1)
    wr = np.concatenate([res["wr_out"] for res in results], axis=1)
    return (r, h, c, mem, wu_new, wlu_new, wr)


def kernel(**inputs):
    from concourse.bass_utils import run_bass_kernel_spmd
    nc = _get_nc(M_FULL)
    in_maps = make_in_maps(inputs, M_FULL)
    out = run_bass_kernel_spmd(nc, in_maps, core_ids=list(range(NCORES)))
    return assemble(out.results)


if __name__ == "__main__":
    nc = build_nc()
    print("built ok:", len(nc.m.functions[0].instructions)
          if hasattr(nc.m.functions[0], "instructions") else "n/a")
